# revision 1
# baseline (speedup 1.0000x reference)
"""JPEG blocking detector on 8 Trainium2 NeuronCores (Bass/Tile).

Full inputs: tgt (32,3,512,512) f32. Output (32,1,512,512) f32 in {0,1}.
Data-parallel: 4 images per core.

Per image (H=W=512, bs=8, thresh=100):
  lum ~ R + (0.587/0.299) G + (0.114/0.299) B            (scale-invariant)
  e_h = |lum[:, w] - lum[:, w+1]|  -> column sums -> phase bins (w%8)
  e_v = |lum[r, :] - lum[r+1, :]|  -> row sums    -> phase bins (r%8)
  flag_k = psum_k/(counts_k*512) > 100*((total-psum_k)/(other_k*512) + 1e-12)
  out[r,w] = maskv[r] OR maskh[w],  maskv[r]=rowflag[r%8]*(r<511), similarly maskh.

Layout: image rows r = t*128+p -> SBUF (partition p, block t in free dim).
  - vertical diffs via PE matmul with a bidiagonal +-1 matrix (partition shift)
  - partition reductions via PE matmuls with ones / one-hot matrices
  - output tile = K=2 matmul: out = maskv*1 + (1-maskv)*maskh
"""

import numpy as np
from contextlib import ExitStack

import ml_dtypes

NCORES = 8
NB = 4          # images per core
P = 128         # partitions
T = 4           # row blocks per image
W = 512
C1 = 0.587 / 0.299
C2 = 0.114 / 0.299

# engine assignment knobs (tuned from traces)
LUM_ENGINES = ("vector", "vector")
EH_SUB_ENGINE = "vector"


def _make_consts():
    # bf16 block (128 x 898): [ones128 | D | D_last | Bmat | e10 | ones512]
    D = np.zeros((128, 128), np.float32)
    for m in range(128):
        D[m, m] = -1.0
        if m + 1 < 128:
            D[m + 1, m] = 1.0
    Dl = D.copy()
    Dl[127, 127] = 0.0
    Bm = np.zeros((128, 128), np.float32)
    Bm[0, 127] = 1.0
    cb = np.zeros((128, 898), np.float32)
    cb[:, 0:1] = 1.0
    cb[:, 1:129] = D
    cb[:, 129:257] = Dl
    cb[:, 257:385] = Bm
    cb[0, 385] = 1.0  # e10: column [1;0] for A_last's p=127 entry
    cb[1, 385] = 0.0
    cb[0, 386:898] = 1.0  # ones512 row (B matmul constant row)
    CB = cb.astype(ml_dtypes.bfloat16)

    # f32 block (128 x 66): [onehot8 | id8 | cA(16) | cB(16) | ones16 | LT2]
    oneh = np.zeros((128, 8), np.float32)
    for p in range(128):
        oneh[p, p % 8] = 1.0
    counts = np.array([64] * 7 + [63], np.float32)
    other = 511.0 - counts
    cA8 = 1.0 / (counts * 512.0)
    cB8 = -100.0 / (other * 512.0)
    cf = np.zeros((128, 66), np.float32)
    cf[:, 0:8] = oneh
    cf[0:8, 8:16] = np.eye(8, dtype=np.float32)
    cf[0:1, 16:32] = np.concatenate([cA8, cA8])[None]
    cf[0:1, 32:48] = np.concatenate([cB8, cB8])[None]
    cf[0:1, 48:64] = 1.0  # ones16 (fe row 1)
    # LT2 (2,2) lhsT: out[0]=1-flags, out[1]=flags  (fe=[flags; ones])
    cf[0, 64] = -1.0
    cf[0, 65] = 1.0
    cf[1, 64] = 1.0
    cf[1, 65] = 0.0
    return CB, cf


def _kernel_body(ctx, tc, out, x, cb, cf):
    import concourse.bass as bass  # noqa: F401
    from concourse import mybir
    from concourse.alu_op_type import AluOpType as alu

    nc = tc.nc
    f32 = mybir.dt.float32
    bf16 = mybir.dt.bfloat16
    Abs = mybir.ActivationFunctionType.Abs
    X = mybir.AxisListType.X

    singles = ctx.enter_context(tc.tile_pool(name="singles", bufs=1))
    pin = ctx.enter_context(tc.tile_pool(name="pin", bufs=4))
    pwork = ctx.enter_context(tc.tile_pool(name="pwork", bufs=3))
    ptiny = ctx.enter_context(tc.tile_pool(name="ptiny", bufs=4))
    posb = ctx.enter_context(tc.tile_pool(name="posb", bufs=2))
    ppsc = ctx.enter_context(tc.tile_pool(name="ppsc", bufs=1, space="PSUM"))
    pevp = ctx.enter_context(tc.tile_pool(name="pevp", bufs=2, space="PSUM"))
    pptiny = ctx.enter_context(tc.tile_pool(name="pptiny", bufs=3, space="PSUM"))
    poutp = ctx.enter_context(tc.tile_pool(name="poutp", bufs=2, space="PSUM"))

    csb = singles.tile([128, 898], bf16, tag="csb")
    nc.sync.dma_start(out=csb, in_=cb)
    csf = singles.tile([128, 66], f32, tag="csf")
    nc.sync.dma_start(out=csf, in_=cf)
    zeros = singles.tile([128, 1], f32, tag="zeros")
    nc.vector.memset(zeros, 0.0)

    ones128 = csb[:, 0:1]
    D = csb[:, 1:129]
    Dl = csb[:, 129:257]
    Bm = csb[:, 257:385]
    ones512 = csb[0:1, 386:898]
    oneh = csf[:, 0:8]
    id8 = csf[0:8, 8:16]
    cA = csf[0:1, 16:32]
    cB = csf[0:1, 32:48]

    eng = lambda name: getattr(nc, name)

    for b in range(NB):
        rgb = pin.tile([P, 3, T, W], bf16, tag="rgb")
        nc.gpsimd.dma_start(out=rgb, in_=x[b].rearrange("c (t p) w -> p c t w", p=P))
        R, G, Bl = rgb[:, 0], rgb[:, 1], rgb[:, 2]

        t1 = pwork.tile([P, T, W], bf16, tag="t1")
        eng(LUM_ENGINES[0]).scalar_tensor_tensor(t1, G, C1, R, alu.mult, alu.add)
        lum = pwork.tile([P, T, W], bf16, tag="lum")
        eng(LUM_ENGINES[1]).scalar_tensor_tensor(lum, Bl, C2, t1, alu.mult, alu.add)

        # horizontal diffs -> per-column sums (over all rows) -> phase bins
        ehs = pwork.tile([P, T, 511], bf16, tag="ehs")
        eng(EH_SUB_ENGINE).tensor_tensor(
            ehs, lum[:, :, 0:511], lum[:, :, 1:512], alu.subtract
        )
        eha = pwork.tile([P, T, W], bf16, tag="eha")
        nc.vector.memset(eha[:, :, 511:512], 0.0)
        nc.scalar.activation(eha[:, :, 0:511], ehs, Abs, bias=zeros)

        psc = ppsc.tile([1, W], f32, tag="psc")
        for t in range(T):
            nc.tensor.matmul(
                psc, lhsT=ones128, rhs=eha[:, t], start=(t == 0), stop=(t == T - 1)
            )

        # vertical diffs via difference-matrix matmuls; row sums via accum_out
        rows = ptiny.tile([P, T], f32, tag="rows")
        for t in range(T):
            evp = pevp.tile([P, W], f32, tag="evp")
            if t < T - 1:
                nc.tensor.matmul(evp, lhsT=D, rhs=lum[:, t], start=True, stop=False)
                nc.tensor.matmul(
                    evp, lhsT=Bm, rhs=lum[:, t + 1], start=False, stop=True
                )
            else:
                nc.tensor.matmul(evp, lhsT=Dl, rhs=lum[:, t], start=True, stop=True)
            scr = pwork.tile([P, W], bf16, tag="scr")
            nc.scalar.activation(
                scr, evp, Abs, bias=zeros, accum_out=rows[:, t : t + 1]
            )

        pph = pptiny.tile([8, T], f32, tag="tinyp")
        nc.tensor.matmul(pph, lhsT=oneh, rhs=rows, start=True, stop=True)
        rowph = ptiny.tile([8, 1], f32, tag="rowph")
        nc.vector.tensor_reduce(rowph, pph, axis=X, op=alu.add)

        ph2 = ptiny.tile([1, 16], f32, tag="ph2")
        nc.vector.tensor_reduce(
            ph2[0:1, 0:8], psc.rearrange("p (i j) -> p j i", j=8), axis=X, op=alu.add
        )
        prt = pptiny.tile([1, 8], f32, tag="tinyp")
        nc.tensor.matmul(prt, lhsT=rowph, rhs=id8, start=True, stop=True)
        nc.scalar.copy(ph2[0:1, 8:16], prt)

        # flags: a_k > thresh*(bg_k + eps)
        tot = ptiny.tile([1, 2], f32, tag="tot")
        nc.vector.tensor_reduce(
            tot, ph2.rearrange("p (g k) -> p g k", g=2), axis=X, op=alu.add
        )
        u = ptiny.tile([1, 16], f32, tag="u")
        nc.vector.tensor_scalar(u[0:1, 0:8], ph2[0:1, 0:8], tot[0:1, 0:1], None, alu.subtract)
        nc.vector.tensor_scalar(u[0:1, 8:16], ph2[0:1, 8:16], tot[0:1, 1:2], None, alu.subtract)
        av = ptiny.tile([1, 16], f32, tag="av")
        nc.vector.tensor_tensor(av, ph2, cA, alu.mult)
        vv = ptiny.tile([1, 16], f32, tag="vv")
        nc.vector.tensor_tensor(vv, u, cB, alu.mult)
        flags = ptiny.tile([1, 16], f32, tag="flags")
        nc.vector.scalar_tensor_tensor(flags, vv, 1e-10, av, alu.add, alu.is_lt)
        nflags = ptiny.tile([1, 16], f32, tag="nflags")
        nc.vector.tensor_scalar(nflags, flags, -1.0, 1.0, alu.mult, alu.add)

        # mask vectors, all on partition 0 (bf16 for the PE rank-1 expansion)
        Amv = ptiny.tile([1, 128], bf16, tag="Amv")  # maskv pattern
        nc.vector.tensor_copy(out=Amv[:, 0:8], in_=flags[0:1, 8:16])
        for sz in (8, 16, 32, 64):
            nc.vector.tensor_copy(out=Amv[:, sz : 2 * sz], in_=Amv[:, 0:sz])
        Anv = ptiny.tile([1, 128], bf16, tag="Anv")  # 1 - maskv
        nc.vector.tensor_copy(out=Anv[:, 0:8], in_=nflags[0:1, 8:16])
        for sz in (8, 16, 32, 64):
            nc.vector.tensor_copy(out=Anv[:, sz : 2 * sz], in_=Anv[:, 0:sz])
        # last-block variants: row 511 excluded -> maskv[127]=0, (1-maskv)[127]=1
        Amvl = ptiny.tile([1, 128], bf16, tag="Amvl")
        nc.vector.tensor_copy(out=Amvl[:, 0:127], in_=Amv[:, 0:127])
        nc.vector.memset(Amvl[:, 127:128], 0.0)
        Anvl = ptiny.tile([1, 128], bf16, tag="Anvl")
        nc.vector.tensor_copy(out=Anvl[:, 0:127], in_=Anv[:, 0:127])
        nc.vector.memset(Anvl[:, 127:128], 1.0)
        # maskh (1,512)
        mh = ptiny.tile([1, W], bf16, tag="mh")
        nc.vector.tensor_copy(out=mh[:, 0:8], in_=flags[0:1, 0:8])
        for sz in (8, 16, 32, 64, 128, 256):
            nc.vector.tensor_copy(out=mh[:, sz : 2 * sz], in_=mh[:, 0:sz])
        nc.vector.memset(mh[:, 511:512], 0.0)

        # out[p,w] = (1-maskv[p])*maskh[w] + maskv[p]*1
        osb = posb.tile([P, T, W], f32, tag="osb")
        for t in range(T):
            op_ = poutp.tile([P, W], f32, tag="outp")
            anv, amv = (Anv, Amv) if t < T - 1 else (Anvl, Amvl)
            nc.tensor.matmul(op_, lhsT=anv, rhs=mh, start=True, stop=False)
            nc.tensor.matmul(op_, lhsT=amv, rhs=ones512, start=False, stop=True)
            nc.scalar.copy(osb[:, t], op_)
        nc.sync.dma_start(out=out[b, 0].rearrange("(t p) w -> p t w", p=P), in_=osb)


_CACHED_NC = None


def _build_nc():
    global _CACHED_NC
    if _CACHED_NC is not None:
        return _CACHED_NC
    import concourse.bass as bass
    import concourse.tile as tile
    from concourse import bacc, mybir

    nc = bacc.Bacc("TRN2", target_bir_lowering=False, debug=False)
    x = nc.dram_tensor("x", [NB, 3, 512, 512], mybir.dt.float32, kind="ExternalInput").ap()
    cb = nc.dram_tensor("cb", [128, 898], mybir.dt.bfloat16, kind="ExternalInput").ap()
    cf = nc.dram_tensor("cf", [128, 66], mybir.dt.float32, kind="ExternalInput").ap()
    out = nc.dram_tensor(
        "out", [NB, 1, 512, 512], mybir.dt.float32, kind="ExternalOutput"
    ).ap()
    with tile.TileContext(nc) as tc, ExitStack() as ctx:
        _kernel_body(ctx, tc, out, x, cb, cf)
    if not nc.is_finalized():
        nc.finalize()
    _CACHED_NC = nc
    return nc


def make_in_maps(tgt):
    CB, CF = _make_consts()
    tgt = np.ascontiguousarray(tgt, dtype=np.float32)
    return [
        {"x": tgt[i * NB : (i + 1) * NB], "cb": CB, "cf": CF} for i in range(NCORES)
    ]


def run(tgt, **kwargs):
    from concourse.bass_utils import run_bass_kernel_spmd

    nc = _build_nc()
    res = run_bass_kernel_spmd(nc, make_in_maps(tgt), core_ids=list(range(NCORES)), **kwargs)
    full = np.concatenate([r["out"] for r in res.results], axis=0)
    return full, res


def kernel(tgt):
    full, _ = run(tgt)
    return full



# revision 6
# speedup vs baseline: 1.1882x; 1.1882x over previous
"""JPEG blocking detector on 8 Trainium2 NeuronCores (Bass/Tile).

Full inputs: tgt (32,3,512,512) f32. Output (32,1,512,512) f32 in {0,1}.
Data-parallel: 4 images per core.

Per image (H=W=512, bs=8, thresh=100):
  lum ~ R + (0.587/0.299) G + (0.114/0.299) B            (scale-invariant)
  e_h = |lum[:, w] - lum[:, w+1]|  -> column sums -> phase bins (w%8)
  e_v = |lum[r, :] - lum[r+1, :]|  -> row sums    -> phase bins (r%8)
  flag_k = psum_k/(counts_k*512) > 100*((total-psum_k)/(other_k*512) + 1e-12)
  out[r,w] = maskv[r] OR maskh[w],  maskv[r]=rowflag[r%8]*(r<511), similarly maskh.

Layout: partition p holds CONSECUTIVE image rows 4p..4p+3 (free dim = (k,w)).
  - vertical diffs are free-dim shifts within a partition; only the
    partition-boundary rows need one shift matmul (S) on the PE
  - column sums via ones-column matmul; abs fused into tensor_reduce
  - input cast f32->bf16 on host (halves HBM read), output fp8 (0/1 exact)
"""

import numpy as np
from contextlib import ExitStack

import ml_dtypes

NCORES = 8
NB = 4          # images per core
P = 128         # partitions
K4 = 4          # rows per partition
W = 512
C1 = 0.587 / 0.299
C2 = 0.114 / 0.299

# engine assignment knobs (tuned from traces). gpsimd is ~2.6 cyc/elem and
# shares the DVE SBUF port -> keep big elementwise off it.
ENG_T1 = "vector"      # t1 = R + c1*G
ENG_LUM = "vector"     # lum = t1 + c2*B
ENG_DH = "vector"      # horizontal diff
ENG_ABS_DH = "scalar"  # |dh|  ("scalar" uses activation Abs, else ts abs_max)
ENG_DVA = "vector"     # vertical diff (in-partition, 3/4 of rows)
ENG_DVB = "vector"     # vertical diff (partition boundary row)
ENG_SRED = "vector"    # row sums of |dv|  (tensor_reduce, vector only)
ENG_OUT = "scalar"     # output expansion ("scalar": activation; else ts max)
IN_DMA_ENGS = ("sync", "sync", "sync", "sync")
OUT_DMA_ENGS = ("scalar", "scalar", "scalar", "scalar")

IN_NPDT = ml_dtypes.bfloat16
OUT_NPDT = ml_dtypes.float8_e4m3


def _make_consts():
    # bf16 block (128 x 129): [S | ones_col]
    #   S[k, m] = 1 iff k == m+1  (bnd[m,:] = lum[m+1,:512]; col 127 = 0)
    S = np.zeros((128, 129), np.float32)
    for m in range(127):
        S[m + 1, m] = 1.0
    S[:, 128] = 1.0
    CBS = S.astype(ml_dtypes.bfloat16)

    # bf16 row block (1 x 384): [ones128 | evenind | oddind]  (K=1 lhsT rows)
    cd = np.zeros((1, 384), np.float32)
    cd[0, 0:128] = 1.0
    cd[0, 128:256] = (np.arange(128) % 2 == 0).astype(np.float32)
    cd[0, 256:384] = (np.arange(128) % 2 == 1).astype(np.float32)
    CD = cd.astype(ml_dtypes.bfloat16)

    # f32 block (128 x 38): [Eev | Eod | cA(16) | cB(16) | rmask(4)]
    cf = np.zeros((128, 38), np.float32)
    cf[:, 0] = (np.arange(128) % 2 == 0).astype(np.float32)
    cf[:, 1] = (np.arange(128) % 2 == 1).astype(np.float32)
    counts = np.array([64] * 7 + [63], np.float32)
    other = 511.0 - counts
    cA8 = 1.0 / (counts * 512.0)
    cB8 = -100.0 / (other * 512.0)
    cf[0, 2:18] = np.concatenate([cA8, cA8])
    cf[0, 18:34] = np.concatenate([cB8, cB8])
    cf[:, 34:38] = 1.0
    cf[127, 37] = 0.0  # row 511 has no vertical diff
    return CBS, CD, cf


def _kernel_body(ctx, tc, out, x, cbs, cd, cf):
    import concourse.bass as bass  # noqa: F401
    from concourse import mybir
    from concourse.alu_op_type import AluOpType as alu

    nc = tc.nc
    f32 = mybir.dt.float32
    bf16 = mybir.dt.bfloat16
    fp8 = mybir.dt.float8e4
    Abs = mybir.ActivationFunctionType.Abs
    Ident = mybir.ActivationFunctionType.Identity
    X = mybir.AxisListType.X

    singles = ctx.enter_context(tc.tile_pool(name="singles", bufs=1))
    pin = ctx.enter_context(tc.tile_pool(name="pin", bufs=2))
    pwork = ctx.enter_context(tc.tile_pool(name="pwork", bufs=2))
    posb = ctx.enter_context(tc.tile_pool(name="posb", bufs=2))
    ptiny = ctx.enter_context(tc.tile_pool(name="ptiny", bufs=2))
    pbnd = ctx.enter_context(tc.tile_pool(name="pbnd", bufs=2, space="PSUM"))
    ppsc = ctx.enter_context(tc.tile_pool(name="ppsc", bufs=2, space="PSUM"))
    pmh = ctx.enter_context(tc.tile_pool(name="pmh", bufs=2, space="PSUM"))
    ptp = ctx.enter_context(tc.tile_pool(name="ptp", bufs=2, space="PSUM"))

    csb = singles.tile([128, 129], bf16, tag="csb")
    nc.sync.dma_start(out=csb, in_=cbs)
    cds = singles.tile([1, 384], bf16, tag="cds")
    nc.sync.dma_start(out=cds, in_=cd)
    csf = singles.tile([128, 38], f32, tag="csf")
    nc.sync.dma_start(out=csf, in_=cf)

    Smat = csb[:, 0:128]
    ones_col = csb[:, 128:129]    # [128,1] lhsT -> M=1 column sums
    ones_row = cds[0:1, 0:128]    # [1,128] lhsT -> K=1 broadcast to 128 parts
    even_row = cds[0:1, 128:256]
    odd_row = cds[0:1, 256:384]
    Eev = csf[:, 0:1]
    Eod = csf[:, 1:2]
    cA = csf[0:1, 2:18]
    cB = csf[0:1, 18:34]
    rmask = csf[:, 34:38]

    eng = lambda name: getattr(nc, name)

    for b in range(NB):
        rgb = pin.tile([P, 3, K4, W], bf16, tag="rgb")
        eng(IN_DMA_ENGS[b]).dma_start(
            out=rgb, in_=x[b].rearrange("c (p k) w -> p c k w", p=P)
        )
        R, G, Bl = rgb[:, 0], rgb[:, 1], rgb[:, 2]

        t1 = pwork.tile([P, K4, W], bf16, tag="t1")
        eng(ENG_T1).scalar_tensor_tensor(t1, G, C1, R, alu.mult, alu.add)
        lum = pwork.tile([P, K4, W], bf16, tag="lum")
        eng(ENG_LUM).scalar_tensor_tensor(lum, Bl, C2, t1, alu.mult, alu.add)

        # horizontal: dh = lum[:,:,w+1]-lum[:,:,w]; |dh| -> column sums via PE
        dha = pwork.tile([P, K4, W], bf16, tag="dha")
        eng(ENG_DH).tensor_tensor(
            dha[:, :, 0:511], lum[:, :, 1:512], lum[:, :, 0:511], alu.subtract
        )
        adh = pwork.tile([P, K4, W], bf16, tag="adh")
        if ENG_ABS_DH == "scalar":
            nc.scalar.activation(adh[:, :, 0:511], dha[:, :, 0:511], Abs)
        else:
            eng(ENG_ABS_DH).tensor_scalar(
                adh[:, :, 0:511], dha[:, :, 0:511], 0.0, None, alu.abs_max
            )
        nc.vector.memset(adh[:, :, 511:512], 0.0)

        psc = ppsc.tile([1, W], f32, tag="psc")
        for k in range(K4):
            nc.tensor.matmul(
                psc, lhsT=ones_col, rhs=adh[:, k], start=(k == 0), stop=(k == 3)
            )

        # vertical: rows 4p..4p+2 diff in-partition; row 4p+3 needs row 4p+4
        # from the next partition -> shift matmul S
        bnd = pbnd.tile([P, W], f32, tag="bnd")
        nc.tensor.matmul(bnd, lhsT=Smat, rhs=lum[:, 0], start=True, stop=True)
        dvt = pwork.tile([P, K4, W], bf16, tag="dvt")
        eng(ENG_DVA).tensor_tensor(
            dvt[:, 0:3], lum[:, 1:4], lum[:, 0:3], alu.subtract
        )
        eng(ENG_DVB).tensor_tensor(dvt[:, 3], bnd, lum[:, 3], alu.subtract)

        s0 = ptiny.tile([P, K4], f32, tag="s0")
        eng(ENG_SRED).tensor_reduce(
            s0, dvt, axis=X, op=alu.add, apply_absolute_value=True
        )
        s = ptiny.tile([P, K4], f32, tag="s")
        nc.vector.tensor_tensor(s, s0, rmask, alu.mult)  # drop row-511 diff

        # phase sums: cols from psc fold; rows via even/odd partition matmuls
        tp = ptp.tile([P, 12], f32, tag="tp")  # [0:1,0:8]=red2, [:,8:12]=mvp
        red2 = tp[0:1, 0:8]
        mvp = tp[:, 8:12]
        nc.tensor.matmul(red2[0:1, 0:4], lhsT=Eev, rhs=s, start=True, stop=True)
        nc.tensor.matmul(red2[0:1, 4:8], lhsT=Eod, rhs=s, start=True, stop=True)

        ph16 = ptiny.tile([1, 16], f32, tag="ph16")
        nc.vector.tensor_reduce(
            ph16[0:1, 0:8], psc.rearrange("p (i j) -> p j i", j=8), axis=X, op=alu.add
        )
        nc.vector.tensor_copy(out=ph16[0:1, 8:16], in_=red2)

        # flags: a_k > thresh*(bg_k + eps)
        tot = ptiny.tile([1, 2], f32, tag="tot")
        nc.vector.tensor_reduce(
            tot, ph16.rearrange("p (g k) -> p g k", g=2), axis=X, op=alu.add
        )
        u = ptiny.tile([1, 16], f32, tag="u")
        nc.vector.tensor_scalar(u[0:1, 0:8], ph16[0:1, 0:8], tot[0:1, 0:1], None, alu.subtract)
        nc.vector.tensor_scalar(u[0:1, 8:16], ph16[0:1, 8:16], tot[0:1, 1:2], None, alu.subtract)
        av = ptiny.tile([1, 16], f32, tag="av")
        nc.vector.tensor_tensor(av, ph16, cA, alu.mult)
        vv = ptiny.tile([1, 16], f32, tag="vv")
        nc.vector.tensor_tensor(vv, u, cB, alu.mult)
        flags = ptiny.tile([1, 16], f32, tag="flags")
        nc.vector.scalar_tensor_tensor(flags, vv, 1e-10, av, alu.add, alu.is_lt)
        flags16 = ptiny.tile([1, 16], bf16, tag="flags16")
        nc.vector.tensor_copy(out=flags16, in_=flags)

        # maskv[p,k] = rowflag[k + 4*(p%2)] via two K=1 matmuls
        nc.tensor.matmul(mvp, lhsT=even_row, rhs=flags16[0:1, 8:12], start=True, stop=False)
        nc.tensor.matmul(mvp, lhsT=odd_row, rhs=flags16[0:1, 12:16], start=False, stop=True)
        mv = ptiny.tile([P, K4], f32, tag="mv")
        nc.vector.tensor_tensor(mv, mvp, rmask, alu.mult)  # row 511 excluded
        nmv = ptiny.tile([P, K4], f32, tag="nmv")
        nc.vector.tensor_scalar(nmv, mv, -1.0, 1.0, alu.mult, alu.add)

        # maskh replicated to all partitions: ones128^T (1x128) @ bcast flags
        mh = pmh.tile([P, W], f32, tag="mh")
        rhs_bcast = flags16[0:1, 0:8].unsqueeze(1).broadcast_to([1, 64, 8])
        nc.tensor.matmul(mh, lhsT=ones_row, rhs=rhs_bcast, start=True, stop=True)
        nc.vector.memset(mh[:, 511:512], 0.0)  # col 511 excluded

        # out[p, k, w] = maskv[p,k] OR maskh[w]
        osb = posb.tile([P, K4, W], fp8, tag="osb")
        for k in range(K4):
            if ENG_OUT == "scalar":
                nc.scalar.activation(
                    osb[:, k], mh, Ident,
                    bias=mv[:, k : k + 1], scale=nmv[:, k : k + 1],
                )
            else:
                eng(ENG_OUT).tensor_scalar(
                    osb[:, k], mh, mv[:, k : k + 1], None, alu.max
                )
        eng(OUT_DMA_ENGS[b]).dma_start(
            out=out[b, 0].rearrange("(p k) w -> p k w", p=P), in_=osb
        )


_CACHED_NC = None


def _build_nc():
    global _CACHED_NC
    if _CACHED_NC is not None:
        return _CACHED_NC
    import concourse.bass as bass
    import concourse.tile as tile
    from concourse import bacc, mybir

    nc = bacc.Bacc("TRN2", target_bir_lowering=False, debug=False)
    x = nc.dram_tensor("x", [NB, 3, 512, 512], mybir.dt.bfloat16, kind="ExternalInput").ap()
    cbs = nc.dram_tensor("cbs", [128, 129], mybir.dt.bfloat16, kind="ExternalInput").ap()
    cd = nc.dram_tensor("cd", [1, 384], mybir.dt.bfloat16, kind="ExternalInput").ap()
    cf = nc.dram_tensor("cf", [128, 38], mybir.dt.float32, kind="ExternalInput").ap()
    out = nc.dram_tensor(
        "out", [NB, 1, 512, 512], mybir.dt.float8e4, kind="ExternalOutput"
    ).ap()
    with tile.TileContext(nc) as tc, ExitStack() as ctx:
        _kernel_body(ctx, tc, out, x, cbs, cd, cf)
    if not nc.is_finalized():
        nc.finalize()
    _CACHED_NC = nc
    return nc


def make_in_maps(tgt):
    CBS, CD, CF = _make_consts()
    tgt16 = np.asarray(tgt, dtype=np.float32).astype(IN_NPDT)
    return [
        {"x": tgt16[i * NB : (i + 1) * NB], "cbs": CBS, "cd": CD, "cf": CF}
        for i in range(NCORES)
    ]


def run(tgt, **kwargs):
    from concourse.bass_utils import run_bass_kernel_spmd

    nc = _build_nc()
    res = run_bass_kernel_spmd(nc, make_in_maps(tgt), core_ids=list(range(NCORES)), **kwargs)
    full = np.concatenate([r["out"] for r in res.results], axis=0).astype(np.float32)
    return full, res


def kernel(tgt):
    full, _ = run(tgt)
    return full


# revision 16
# speedup vs baseline: 1.3758x; 1.1579x over previous
"""JPEG blocking detector on 8 Trainium2 NeuronCores (Bass/Tile).

Full inputs: tgt (32,3,512,512) f32. Output (32,1,512,512) f32 in {0,1}.
Data-parallel: 4 images per core.

Per image (H=W=512, bs=8, thresh=100):
  lum ~ R + (0.587/0.299) G + (0.114/0.299) B            (scale-invariant)
  e_h = |lum[:, w] - lum[:, w+1]|  -> column sums -> phase bins (w%8)
  e_v = |lum[r, :] - lum[r+1, :]|  -> row sums    -> phase bins (r%8)
  flag_k = psum_k/(counts_k*512) > 100*(total-psum_k)/(other_k*512)
  out[r,w] = maskv[r] OR maskh[w],  maskv[r]=rowflag[r%8]*(r<511), similarly maskh.

Layout: partition p holds CONSECUTIVE image rows 4p..4p+3 (free dim = (k,w)).
  - host pre-scales channels by luma weights, casts to bf16 (halves HBM read)
  - vertical diffs are free-dim shifts in a partition; boundary rows from one
    PE matmul pair (S*lum0 - I'*lum3); |dh| on the scalar engine
  - row sums via scalar Abs+accumulator / vector reduce; column sums via
    ones-column PE matmuls into a pair-shared PSUM strip (N=511, cols 511
    pre-zeroed once)
  - flag algebra batched per image-PAIR on one partition:
      flag  <=>  (-cB)*tot < ph*(cA-cB)   (eps dropped: strict compare)
  - output fp8 (0/1 exact), upcast on host
"""

import numpy as np
from contextlib import ExitStack

import ml_dtypes

NCORES = 8
NB = 4          # images per core
P = 128         # partitions
K4 = 4          # rows per partition
W = 512
C1 = 0.587 / 0.299
C2 = 0.114 / 0.299

IN_NPDT = ml_dtypes.bfloat16
OUT_NPDT = ml_dtypes.float8_e4m3

# balance knobs: dv row-sum segments on vector (rest scalar); output k's on
# vector ts-max (rest scalar activation)
DV_V_KS = (0,)
OUT_V_KS = (2, 3)


def _make_consts():
    # bf16 block (128 x 257): [S | negI' | ones_col]
    cb = np.zeros((128, 257), np.float32)
    for m in range(127):
        cb[m + 1, m] = 1.0
        cb[m, 128 + m] = -1.0
    cb[:, 256] = 1.0
    CBS = cb.astype(ml_dtypes.bfloat16)

    # bf16 row block (1 x 512): [ones128 | evenind | oddind | odd127z]
    cd = np.zeros((1, 512), np.float32)
    cd[0, 0:128] = 1.0
    cd[0, 128:256] = (np.arange(128) % 2 == 0).astype(np.float32)
    cd[0, 256:384] = (np.arange(128) % 2 == 1).astype(np.float32)
    cd[0, 384:512] = cd[0, 256:384]
    cd[0, 511] = 0.0  # odd127z: excludes (p=127,k=3) i.e. image row 511
    CD = cd.astype(ml_dtypes.bfloat16)

    # f32 col block (128 x 2): [Eev | Eod] for row-phase matmuls (rhs s is f32)
    ce = np.zeros((128, 2), np.float32)
    ce[:, 0] = (np.arange(128) % 2 == 0).astype(np.float32)
    ce[:, 1] = (np.arange(128) % 2 == 1).astype(np.float32)

    # f32 row block (1 x 64): [cAB32 | negcB32]  (per-pair flag algebra)
    counts = np.array([64] * 7 + [63], np.float32)
    other = 511.0 - counts
    cA8 = 1.0 / (counts * 512.0)
    cB8 = -100.0 / (other * 512.0)
    cA16 = np.concatenate([cA8, cA8])
    cB16 = np.concatenate([cB8, cB8])
    cf = np.zeros((1, 64), np.float32)
    cf[0, 0:32] = np.concatenate([cA16 - cB16, cA16 - cB16])
    cf[0, 32:64] = np.concatenate([-cB16, -cB16])
    return CBS, CD, ce, cf


def _kernel_body(ctx, tc, out, x, cbs, cd, ce, cf):
    import concourse.bass as bass  # noqa: F401
    from concourse import mybir
    from concourse.alu_op_type import AluOpType as alu

    nc = tc.nc
    f32 = mybir.dt.float32
    bf16 = mybir.dt.bfloat16
    fp8 = mybir.dt.float8e4
    Abs = mybir.ActivationFunctionType.Abs
    Ident = mybir.ActivationFunctionType.Identity
    Copy = mybir.ActivationFunctionType.Copy
    X = mybir.AxisListType.X

    singles = ctx.enter_context(tc.tile_pool(name="singles", bufs=1))
    pin = ctx.enter_context(tc.tile_pool(name="pin", bufs=2))
    pwork = ctx.enter_context(tc.tile_pool(name="pwork", bufs=2))
    posb = ctx.enter_context(tc.tile_pool(name="posb", bufs=2))
    ptiny = ctx.enter_context(tc.tile_pool(name="ptiny", bufs=2))
    ppair = ctx.enter_context(tc.tile_pool(name="ppair", bufs=2))
    pbnd = ctx.enter_context(tc.tile_pool(name="pbnd", bufs=2, space="PSUM"))
    ppsc = ctx.enter_context(tc.tile_pool(name="ppsc", bufs=1, space="PSUM"))
    pmh = ctx.enter_context(tc.tile_pool(name="pmh", bufs=1, space="PSUM"))
    ptp = ctx.enter_context(tc.tile_pool(name="ptp", bufs=2, space="PSUM"))

    csb = singles.tile([128, 257], bf16, tag="csb")
    nc.sync.dma_start(out=csb, in_=cbs)
    cds = singles.tile([1, 512], bf16, tag="cds")
    nc.sync.dma_start(out=cds, in_=cd)
    cse = singles.tile([128, 2], f32, tag="cse")
    nc.sync.dma_start(out=cse, in_=ce)
    csf = singles.tile([1, 64], f32, tag="csf")
    nc.sync.dma_start(out=csf, in_=cf)

    Smat = csb[:, 0:128]
    negI = csb[:, 128:256]
    ones_col = csb[:, 256:257]
    ones_row = cds[0:1, 0:128]
    even_row = cds[0:1, 128:256]
    odd_row = cds[0:1, 256:384]
    odd127z = cds[0:1, 384:512]
    Eev = cse[:, 0:1]
    Eod = cse[:, 1:2]
    cAB32 = csf[0:1, 0:32]
    negcB32 = csf[0:1, 32:64]

    # pair-shared PSUM column-sum strip; cols 511 of each image never written
    # by the N=511 matmuls -> zero once
    psc = ppsc.tile([1, 2, 512], f32, tag="psc")
    nc.vector.memset(psc[:, :, 511:512], 0.0)

    # mh PSUM tiles: col 511 never written (N=504 + N=7 matmuls) -> zero both
    mh_tiles = []
    for mi in range(2):
        mh0 = pmh.tile([P, W], f32, tag=f"mh{mi}")
        nc.vector.memset(mh0[:, 511:512], 0.0)
        mh_tiles.append(mh0)

    eng = lambda name: getattr(nc, name)

    def stats_phase(b, ph32):
        pj = b % 2
        rgb = pin.tile([P, 3, K4, W], bf16, tag="rgb")
        nc.sync.dma_start(out=rgb, in_=x[b].rearrange("c (p k) w -> p c k w", p=P))

        t1 = pwork.tile([P, K4, W], bf16, tag="t1")
        nc.vector.tensor_tensor(t1, rgb[:, 0], rgb[:, 1], alu.add)
        lum = pwork.tile([P, K4, W], bf16, tag="lum")
        nc.vector.tensor_tensor(lum, t1, rgb[:, 2], alu.add)

        # horizontal |dh| -> PE column sums into psc strip (cols 0..510)
        dha = pwork.tile([P, K4, W], bf16, tag="dha")
        nc.vector.tensor_tensor(
            dha[:, :, 0:511], lum[:, :, 1:512], lum[:, :, 0:511], alu.subtract
        )
        adh = pwork.tile([P, K4, W], bf16, tag="adh")
        nc.scalar.activation(adh[:, :, 0:511], dha[:, :, 0:511], Abs)
        for k in range(K4):
            nc.tensor.matmul(
                psc[0:1, pj, 0:511], lhsT=ones_col, rhs=adh[:, k, 0:511],
                start=(k == 0), stop=(k == 3),
            )

        # vertical: in-partition diffs + PE boundary pair
        bnd = pbnd.tile([P, W], f32, tag="bnd")
        nc.tensor.matmul(bnd, lhsT=Smat, rhs=lum[:, 0], start=True, stop=False)
        nc.tensor.matmul(bnd, lhsT=negI, rhs=lum[:, 3], start=False, stop=True)
        dvt = pwork.tile([P, 3, W], bf16, tag="dvt")
        nc.vector.tensor_tensor(dvt, lum[:, 1:4], lum[:, 0:3], alu.subtract)

        s = ptiny.tile([P, K4], f32, tag="s")
        scrap = pwork.tile([P, K4, W], bf16, tag="scrap")
        for k in range(3):
            if k in DV_V_KS:
                nc.vector.tensor_reduce(
                    s[:, k : k + 1], dvt[:, k : k + 1], axis=X, op=alu.add,
                    apply_absolute_value=True,
                )
            else:
                nc.scalar.activation(
                    scrap[:, k], dvt[:, k], Abs, accum_out=s[:, k : k + 1]
                )
        nc.scalar.activation(scrap[:, 3], bnd, Abs, accum_out=s[:, 3:4])

        # row-phase partials: red2 = [even(4) | odd(4)] partition sums of s
        tp = ptp.tile([P, 12], f32, tag="tp")
        red2 = tp[0:1, 0:8]
        nc.tensor.matmul(red2[0:1, 0:4], lhsT=Eev, rhs=s, start=True, stop=True)
        nc.tensor.matmul(red2[0:1, 4:8], lhsT=Eod, rhs=s, start=True, stop=True)
        # rows: phase j = k + 4*(p%2): [ev k=0..3 -> ph 0..3 | od -> ph 4..7]
        nc.scalar.copy(ph32[0:1, 16 * pj + 8 : 16 * pj + 16], red2)
        return tp

    def flags_phase(ph32):
        # column phases: fold psc strip (both images) into ph32[0:8]/[16:24]
        nc.vector.tensor_reduce(
            ph32.rearrange("p (a x) -> p a x", a=2)[:, :, 0:8],
            psc.rearrange("p a (i j) -> p a j i", j=8),
            axis=X, op=alu.add,
        )
        tot4 = ppair.tile([1, 4], f32, tag="tot4")
        nc.vector.tensor_reduce(
            tot4, ph32.rearrange("p (g j) -> p g j", j=8), axis=X, op=alu.add
        )
        q = ppair.tile([1, 32], f32, tag="q")
        nc.vector.tensor_tensor(q, ph32, cAB32, alu.mult)
        v2 = ppair.tile([1, 32], f32, tag="v2")
        tot_b = tot4.unsqueeze(2).broadcast_to([1, 4, 8])
        nc.vector.tensor_tensor(
            v2.rearrange("p (g j) -> p g j", j=8), negcB32.rearrange("p (g j) -> p g j", j=8),
            tot_b, alu.mult,
        )
        flagsP = ppair.tile([1, 32], f32, tag="flagsP")
        nc.vector.tensor_tensor(flagsP, v2, q, alu.is_lt)
        flags16 = ppair.tile([1, 32], bf16, tag="flags16")
        nc.scalar.copy(flags16, flagsP)
        return flags16

    def out_phase(b, flags16, tp):
        pj = b % 2
        fr = flags16[0:1, 16 * pj : 16 * pj + 16]
        # maskv[p,k] = rowflag[k+4*(p%2)], with (127,3) zeroed via odd127z
        mvp = tp[:, 8:12]
        nc.tensor.matmul(mvp[:, 0:3], lhsT=even_row, rhs=fr[0:1, 8:11], start=True, stop=False)
        nc.tensor.matmul(mvp[:, 0:3], lhsT=odd_row, rhs=fr[0:1, 12:15], start=False, stop=True)
        nc.tensor.matmul(mvp[:, 3:4], lhsT=even_row, rhs=fr[0:1, 11:12], start=True, stop=False)
        nc.tensor.matmul(mvp[:, 3:4], lhsT=odd127z, rhs=fr[0:1, 15:16], start=False, stop=True)
        mv = ptiny.tile([P, K4], f32, tag="mv")
        nc.scalar.copy(mv, mvp)
        nmv = ptiny.tile([P, K4], f32, tag="nmv")
        nc.scalar.activation(nmv, mvp, Copy, bias=1.0, scale=-1.0)

        # maskh replicated to all partitions; col 511 stays 0 (pre-zeroed)
        mh = mh_tiles[b % 2]
        bc = fr[0:1, 0:8].unsqueeze(1)
        nc.tensor.matmul(
            mh[:, 0:504], lhsT=ones_row, rhs=bc.broadcast_to([1, 63, 8]),
            start=True, stop=True,
        )
        nc.tensor.matmul(
            mh[:, 504:511], lhsT=ones_row, rhs=fr[0:1, 0:7],
            start=True, stop=True,
        )

        osb = posb.tile([P, K4, W], fp8, tag="osb")
        for k in range(K4):
            if k in OUT_V_KS:
                nc.vector.tensor_scalar(
                    osb[:, k], mh, mvp[:, k : k + 1], None, alu.max
                )
            else:
                nc.scalar.activation(
                    osb[:, k], mh, Ident,
                    bias=mv[:, k : k + 1], scale=nmv[:, k : k + 1],
                )
        nc.sync.dma_start(
            out=out[b, 0].rearrange("(p k) w -> p k w", p=P), in_=osb
        )

    for pi in range(2):
        ph32 = ppair.tile([1, 32], f32, tag="ph32")
        tps = [stats_phase(2 * pi + j, ph32) for j in range(2)]
        flags16 = flags_phase(ph32)
        for j in range(2):
            out_phase(2 * pi + j, flags16, tps[j])


_CACHED_NC = None


def _build_nc():
    global _CACHED_NC
    if _CACHED_NC is not None:
        return _CACHED_NC
    import concourse.bass as bass
    import concourse.tile as tile
    from concourse import bacc, mybir

    nc = bacc.Bacc("TRN2", target_bir_lowering=False, debug=False)
    x = nc.dram_tensor("x", [NB, 3, 512, 512], mybir.dt.bfloat16, kind="ExternalInput").ap()
    cbs = nc.dram_tensor("cbs", [128, 257], mybir.dt.bfloat16, kind="ExternalInput").ap()
    cd = nc.dram_tensor("cd", [1, 512], mybir.dt.bfloat16, kind="ExternalInput").ap()
    ce = nc.dram_tensor("ce", [128, 2], mybir.dt.float32, kind="ExternalInput").ap()
    cf = nc.dram_tensor("cf", [1, 64], mybir.dt.float32, kind="ExternalInput").ap()
    out = nc.dram_tensor(
        "out", [NB, 1, 512, 512], mybir.dt.float8e4, kind="ExternalOutput"
    ).ap()
    with tile.TileContext(nc) as tc, ExitStack() as ctx:
        _kernel_body(ctx, tc, out, x, cbs, cd, ce, cf)
    if not nc.is_finalized():
        nc.finalize()
    _CACHED_NC = nc
    return nc


def make_in_maps(tgt):
    CBS, CD, CE, CF = _make_consts()
    tgt32 = np.asarray(tgt, dtype=np.float32)
    wch = np.array([1.0, C1, C2], np.float32).reshape(1, 3, 1, 1)
    tgt16 = (tgt32 * wch).astype(IN_NPDT)
    return [
        {"x": tgt16[i * NB : (i + 1) * NB], "cbs": CBS, "cd": CD, "ce": CE, "cf": CF}
        for i in range(NCORES)
    ]


def run(tgt, **kwargs):
    from concourse.bass_utils import run_bass_kernel_spmd

    nc = _build_nc()
    res = run_bass_kernel_spmd(nc, make_in_maps(tgt), core_ids=list(range(NCORES)), **kwargs)
    full = np.concatenate([r["out"] for r in res.results], axis=0).astype(np.float32)
    return full, res


def kernel(tgt):
    full, _ = run(tgt)
    return full


# revision 17
# speedup vs baseline: 1.4653x; 1.0650x over previous
"""JPEG blocking detector on 8 Trainium2 NeuronCores (Bass/Tile).

Full inputs: tgt (32,3,512,512) f32. Output (32,1,512,512) f32 in {0,1}.
Data-parallel: 4 images per core.

Per image (H=W=512, bs=8, thresh=100):
  lum ~ R + (0.587/0.299) G + (0.114/0.299) B            (scale-invariant)
  e_h = |lum[:, w] - lum[:, w+1]|  -> column sums -> phase bins (w%8)
  e_v = |lum[r, :] - lum[r+1, :]|  -> row sums    -> phase bins (r%8)
  flag_k = psum_k/(counts_k*512) > 100*(total-psum_k)/(other_k*512)
  out[r,w] = maskv[r] OR maskh[w],  maskv[r]=rowflag[r%8]*(r<511), similarly maskh.

Layout: partition p holds CONSECUTIVE image rows 4p..4p+3 (free dim = (k,w)).
  - host pre-scales channels by luma weights, casts to bf16 (halves HBM read)
  - vertical diffs are free-dim shifts in a partition; boundary rows from one
    PE matmul pair (S*lum0 - I'*lum3); |dh| on the scalar engine
  - row sums via scalar Abs+accumulator / vector reduce; column sums via
    ones-column PE matmuls into a pair-shared PSUM strip (N=511, cols 511
    pre-zeroed once)
  - flag algebra batched per image-PAIR on one partition:
      flag  <=>  (-cB)*tot < ph*(cA-cB)   (eps dropped: strict compare)
  - output fp8 (0/1 exact), upcast on host
"""

import numpy as np
from contextlib import ExitStack

import ml_dtypes

NCORES = 8
NB = 4          # images per core
P = 128         # partitions
K4 = 4          # rows per partition
W = 512
C1 = 0.587 / 0.299
C2 = 0.114 / 0.299

IN_NPDT = ml_dtypes.bfloat16
OUT_NPDT = ml_dtypes.float8_e4m3

# balance knobs: dv row-sum segments on vector (rest scalar); output k's on
# vector ts-max (rest scalar activation)
DV_V_KS = (0,)
OUT_V_KS = (2, 3)


def _make_consts():
    # bf16 block (128 x 257): [S | negI' | ones_col]
    cb = np.zeros((128, 257), np.float32)
    for m in range(127):
        cb[m + 1, m] = 1.0
        cb[m, 128 + m] = -1.0
    cb[:, 256] = 1.0
    CBS = cb.astype(ml_dtypes.bfloat16)

    # bf16 row block (1 x 512): [ones128 | evenind | oddind | odd127z]
    cd = np.zeros((1, 512), np.float32)
    cd[0, 0:128] = 1.0
    cd[0, 128:256] = (np.arange(128) % 2 == 0).astype(np.float32)
    cd[0, 256:384] = (np.arange(128) % 2 == 1).astype(np.float32)
    cd[0, 384:512] = cd[0, 256:384]
    cd[0, 511] = 0.0  # odd127z: excludes (p=127,k=3) i.e. image row 511
    CD = cd.astype(ml_dtypes.bfloat16)

    # f32 col block (128 x 2): [Eev | Eod] for row-phase matmuls (rhs s is f32)
    ce = np.zeros((128, 2), np.float32)
    ce[:, 0] = (np.arange(128) % 2 == 0).astype(np.float32)
    ce[:, 1] = (np.arange(128) % 2 == 1).astype(np.float32)

    # f32 row block (1 x 64): [cAB32 | negcB32]  (per-pair flag algebra)
    counts = np.array([64] * 7 + [63], np.float32)
    other = 511.0 - counts
    cA8 = 1.0 / (counts * 512.0)
    cB8 = -100.0 / (other * 512.0)
    cA16 = np.concatenate([cA8, cA8])
    cB16 = np.concatenate([cB8, cB8])
    cf = np.zeros((1, 64), np.float32)
    cf[0, 0:32] = np.concatenate([cA16 - cB16, cA16 - cB16])
    cf[0, 32:64] = np.concatenate([-cB16, -cB16])
    return CBS, CD, ce, cf


def _kernel_body(ctx, tc, out, x, cbs, cd, ce, cf):
    import concourse.bass as bass  # noqa: F401
    from concourse import mybir
    from concourse.alu_op_type import AluOpType as alu

    nc = tc.nc
    f32 = mybir.dt.float32
    bf16 = mybir.dt.bfloat16
    fp8 = mybir.dt.float8e4
    Abs = mybir.ActivationFunctionType.Abs
    Ident = mybir.ActivationFunctionType.Identity
    Copy = mybir.ActivationFunctionType.Copy
    X = mybir.AxisListType.X

    singles = ctx.enter_context(tc.tile_pool(name="singles", bufs=1))
    pin = ctx.enter_context(tc.tile_pool(name="pin", bufs=3))
    pwork = ctx.enter_context(tc.tile_pool(name="pwork", bufs=3))
    posb = ctx.enter_context(tc.tile_pool(name="posb", bufs=3))
    ptiny = ctx.enter_context(tc.tile_pool(name="ptiny", bufs=4))
    ppair = ctx.enter_context(tc.tile_pool(name="ppair", bufs=2))
    pbnd = ctx.enter_context(tc.tile_pool(name="pbnd", bufs=2, space="PSUM"))
    ppsc = ctx.enter_context(tc.tile_pool(name="ppsc", bufs=1, space="PSUM"))
    pmh = ctx.enter_context(tc.tile_pool(name="pmh", bufs=1, space="PSUM"))
    ptp = ctx.enter_context(tc.tile_pool(name="ptp", bufs=2, space="PSUM"))

    csb = singles.tile([128, 257], bf16, tag="csb")
    nc.gpsimd.dma_start(out=csb, in_=cbs)
    cds = singles.tile([1, 512], bf16, tag="cds")
    nc.gpsimd.dma_start(out=cds, in_=cd)
    cse = singles.tile([128, 2], f32, tag="cse")
    nc.gpsimd.dma_start(out=cse, in_=ce)
    csf = singles.tile([1, 64], f32, tag="csf")
    nc.gpsimd.dma_start(out=csf, in_=cf)

    Smat = csb[:, 0:128]
    negI = csb[:, 128:256]
    ones_col = csb[:, 256:257]
    ones_row = cds[0:1, 0:128]
    even_row = cds[0:1, 128:256]
    odd_row = cds[0:1, 256:384]
    odd127z = cds[0:1, 384:512]
    Eev = cse[:, 0:1]
    Eod = cse[:, 1:2]
    cAB32 = csf[0:1, 0:32]
    negcB32 = csf[0:1, 32:64]

    # pair-shared PSUM column-sum strip; cols 511 of each image never written
    # by the N=511 matmuls -> zero once
    psc = ppsc.tile([1, 2, 512], f32, tag="psc")
    nc.vector.memset(psc[:, :, 511:512], 0.0)

    # mh PSUM tiles: col 511 never written (N=504 + N=7 matmuls) -> zero both
    mh_tiles = []
    for mi in range(2):
        mh0 = pmh.tile([P, W], f32, tag=f"mh{mi}")
        nc.vector.memset(mh0[:, 511:512], 0.0)
        mh_tiles.append(mh0)

    eng = lambda name: getattr(nc, name)

    def stats_phase(b, ph32):
        pj = b % 2
        rgb = pin.tile([P, 3, K4, W], bf16, tag="rgb")
        nc.sync.dma_start(out=rgb, in_=x[b].rearrange("c (p k) w -> p c k w", p=P))

        t1 = pwork.tile([P, K4, W], bf16, tag="t1")
        nc.vector.tensor_tensor(t1, rgb[:, 0], rgb[:, 1], alu.add)
        lum = pwork.tile([P, K4, W], bf16, tag="lum")
        nc.vector.tensor_tensor(lum, t1, rgb[:, 2], alu.add)

        # horizontal |dh| -> PE column sums into psc strip (cols 0..510)
        dha = pwork.tile([P, K4, W], bf16, tag="dha")
        nc.vector.tensor_tensor(
            dha[:, :, 0:511], lum[:, :, 1:512], lum[:, :, 0:511], alu.subtract
        )
        adh = pwork.tile([P, K4, W], bf16, tag="adh")
        nc.scalar.activation(adh[:, :, 0:511], dha[:, :, 0:511], Abs)
        for k in range(K4):
            nc.tensor.matmul(
                psc[0:1, pj, 0:511], lhsT=ones_col, rhs=adh[:, k, 0:511],
                start=(k == 0), stop=(k == 3),
            )

        # vertical: in-partition diffs + PE boundary pair
        bnd = pbnd.tile([P, W], f32, tag="bnd")
        nc.tensor.matmul(bnd, lhsT=Smat, rhs=lum[:, 0], start=True, stop=False)
        nc.tensor.matmul(bnd, lhsT=negI, rhs=lum[:, 3], start=False, stop=True)
        dvt = pwork.tile([P, 3, W], bf16, tag="dvt")
        nc.vector.tensor_tensor(dvt, lum[:, 1:4], lum[:, 0:3], alu.subtract)

        s = ptiny.tile([P, K4], f32, tag="s")
        scrap = pwork.tile([P, K4, W], bf16, tag="scrap")
        for k in range(3):
            if k in DV_V_KS:
                nc.vector.tensor_reduce(
                    s[:, k : k + 1], dvt[:, k : k + 1], axis=X, op=alu.add,
                    apply_absolute_value=True,
                )
            else:
                nc.scalar.activation(
                    scrap[:, k], dvt[:, k], Abs, accum_out=s[:, k : k + 1]
                )
        nc.scalar.activation(scrap[:, 3], bnd, Abs, accum_out=s[:, 3:4])

        # row-phase partials: red2 = [even(4) | odd(4)] partition sums of s
        tp = ptp.tile([P, 12], f32, tag="tp")
        red2 = tp[0:1, 0:8]
        nc.tensor.matmul(red2[0:1, 0:4], lhsT=Eev, rhs=s, start=True, stop=True)
        nc.tensor.matmul(red2[0:1, 4:8], lhsT=Eod, rhs=s, start=True, stop=True)
        # rows: phase j = k + 4*(p%2): [ev k=0..3 -> ph 0..3 | od -> ph 4..7]
        nc.scalar.copy(ph32[0:1, 16 * pj + 8 : 16 * pj + 16], red2)
        return tp

    def flags_phase(ph32):
        # column phases: fold psc strip (both images) into ph32[0:8]/[16:24]
        nc.vector.tensor_reduce(
            ph32.rearrange("p (a x) -> p a x", a=2)[:, :, 0:8],
            psc.rearrange("p a (i j) -> p a j i", j=8),
            axis=X, op=alu.add,
        )
        tot4 = ppair.tile([1, 4], f32, tag="tot4")
        nc.vector.tensor_reduce(
            tot4, ph32.rearrange("p (g j) -> p g j", j=8), axis=X, op=alu.add
        )
        q = ppair.tile([1, 32], f32, tag="q")
        nc.vector.tensor_tensor(q, ph32, cAB32, alu.mult)
        v2 = ppair.tile([1, 32], f32, tag="v2")
        tot_b = tot4.unsqueeze(2).broadcast_to([1, 4, 8])
        nc.vector.tensor_tensor(
            v2.rearrange("p (g j) -> p g j", j=8), negcB32.rearrange("p (g j) -> p g j", j=8),
            tot_b, alu.mult,
        )
        flagsP = ppair.tile([1, 32], f32, tag="flagsP")
        nc.vector.tensor_tensor(flagsP, v2, q, alu.is_lt)
        flags16 = ppair.tile([1, 32], bf16, tag="flags16")
        nc.scalar.copy(flags16, flagsP)
        return flags16

    def out_phase(b, flags16, tp):
        pj = b % 2
        fr = flags16[0:1, 16 * pj : 16 * pj + 16]
        # maskv[p,k] = rowflag[k+4*(p%2)], with (127,3) zeroed via odd127z
        mvp = tp[:, 8:12]
        nc.tensor.matmul(mvp[:, 0:3], lhsT=even_row, rhs=fr[0:1, 8:11], start=True, stop=False)
        nc.tensor.matmul(mvp[:, 0:3], lhsT=odd_row, rhs=fr[0:1, 12:15], start=False, stop=True)
        nc.tensor.matmul(mvp[:, 3:4], lhsT=even_row, rhs=fr[0:1, 11:12], start=True, stop=False)
        nc.tensor.matmul(mvp[:, 3:4], lhsT=odd127z, rhs=fr[0:1, 15:16], start=False, stop=True)
        mv = ptiny.tile([P, K4], f32, tag="mv")
        nc.scalar.copy(mv, mvp)
        nmv = ptiny.tile([P, K4], f32, tag="nmv")
        nc.scalar.activation(nmv, mvp, Copy, bias=1.0, scale=-1.0)

        # maskh replicated to all partitions; col 511 stays 0 (pre-zeroed)
        mh = mh_tiles[b % 2]
        bc = fr[0:1, 0:8].unsqueeze(1)
        nc.tensor.matmul(
            mh[:, 0:504], lhsT=ones_row, rhs=bc.broadcast_to([1, 63, 8]),
            start=True, stop=True,
        )
        nc.tensor.matmul(
            mh[:, 504:511], lhsT=ones_row, rhs=fr[0:1, 0:7],
            start=True, stop=True,
        )

        osb = posb.tile([P, K4, W], fp8, tag="osb")
        for k in range(K4):
            if k in OUT_V_KS:
                nc.vector.tensor_scalar(
                    osb[:, k], mh, mvp[:, k : k + 1], None, alu.max
                )
            else:
                nc.scalar.activation(
                    osb[:, k], mh, Ident,
                    bias=mv[:, k : k + 1], scale=nmv[:, k : k + 1],
                )
        nc.gpsimd.dma_start(
            out=out[b, 0].rearrange("(p k) w -> p k w", p=P), in_=osb
        )

    for pi in range(2):
        ph32 = ppair.tile([1, 32], f32, tag="ph32")
        tps = [stats_phase(2 * pi + j, ph32) for j in range(2)]
        flags16 = flags_phase(ph32)
        for j in range(2):
            out_phase(2 * pi + j, flags16, tps[j])


_CACHED_NC = None


def _build_nc():
    global _CACHED_NC
    if _CACHED_NC is not None:
        return _CACHED_NC
    import concourse.bass as bass
    import concourse.tile as tile
    from concourse import bacc, mybir

    nc = bacc.Bacc("TRN2", target_bir_lowering=False, debug=False)
    x = nc.dram_tensor("x", [NB, 3, 512, 512], mybir.dt.bfloat16, kind="ExternalInput").ap()
    cbs = nc.dram_tensor("cbs", [128, 257], mybir.dt.bfloat16, kind="ExternalInput").ap()
    cd = nc.dram_tensor("cd", [1, 512], mybir.dt.bfloat16, kind="ExternalInput").ap()
    ce = nc.dram_tensor("ce", [128, 2], mybir.dt.float32, kind="ExternalInput").ap()
    cf = nc.dram_tensor("cf", [1, 64], mybir.dt.float32, kind="ExternalInput").ap()
    out = nc.dram_tensor(
        "out", [NB, 1, 512, 512], mybir.dt.float8e4, kind="ExternalOutput"
    ).ap()
    with tile.TileContext(nc) as tc, ExitStack() as ctx:
        _kernel_body(ctx, tc, out, x, cbs, cd, ce, cf)
    if not nc.is_finalized():
        nc.finalize()
    _CACHED_NC = nc
    return nc


def make_in_maps(tgt):
    CBS, CD, CE, CF = _make_consts()
    tgt32 = np.asarray(tgt, dtype=np.float32)
    wch = np.array([1.0, C1, C2], np.float32).reshape(1, 3, 1, 1)
    tgt16 = (tgt32 * wch).astype(IN_NPDT)
    return [
        {"x": tgt16[i * NB : (i + 1) * NB], "cbs": CBS, "cd": CD, "ce": CE, "cf": CF}
        for i in range(NCORES)
    ]


def run(tgt, **kwargs):
    from concourse.bass_utils import run_bass_kernel_spmd

    nc = _build_nc()
    res = run_bass_kernel_spmd(nc, make_in_maps(tgt), core_ids=list(range(NCORES)), **kwargs)
    full = np.concatenate([r["out"] for r in res.results], axis=0).astype(np.float32)
    return full, res


def kernel(tgt):
    full, _ = run(tgt)
    return full
